# revision 8
# baseline (speedup 1.0000x reference)
"""AnyPrecisionLinear (4-bit LUT-quantized linear) on 8 TRN2 NeuronCores.

Reference computes:  out = x @ W.T,  W[o,i] = lut[o, qweight[o,i]]
  x: [64, 8192] fp16, qweight: [8192, 8192] int32 (values 0..15),
  lut: [8192, 16] fp16  ->  out: [64, 8192] fp16

Strategy (tensor-parallel along out_features, per the sharding hint):
  * Host re-encodes each row's 16-entry LUT into fp8 e3m4 (FP8_EXP3) code
    values with a per-row scale s[o] = max|lut|/15, then gathers per-element
    codes so each device receives a [128, 64*1024] fp8 weight image
    (1 byte/weight; encode rel err measured 1.09e-2 vs the 2e-2 threshold).
  * The TensorEngine consumes fp8e3 rhs directly against the fp16 x lhsT
    (mixed-dtype matmul, HW-verified bit-exact) -- NO on-device dequant.
    The kernel is a pure DMA-stream + matmul-chase pipeline.
  * All transfers use full 128-partition tiles: partition-sliced DMAs
    measured HALF bandwidth (fractional partition/engine split).
  * x head + first weight chunk issue from the Scalar HWDGE ring, which
    reaches user code ~1.2us before the Sync ring; the rest of the weight
    stream rides the Sync ring.  The PE chases chunk semaphores; no PE
    warmup is needed since the DMA stream, not the PE, is critical
    (cold-HAM matmuls hide behind the stream and warm up by chunk ~3).
  * Column-tiled matmul pairs (PSUM partitions 0-63 / 64-127) accumulate
    x @ codes.T; tiny tail chunks limit the consume-lag after the last
    chunk semaphore; chain 1 closes before chain 2 so its epilogue and
    output DMA overlap chain 2's last matmuls.
"""

import numpy as np
import ml_dtypes

import concourse.bass as bass
from concourse import bacc, mybir
from concourse.bass_utils import run_bass_kernel_spmd

B, IN, OUT, NCORES = 64, 8192, 8192, 8
OSH = OUT // NCORES          # 1024 output columns per core
KT = IN // 128               # 64 contraction tiles of 128
CHUNKS = (4, 8, 8, 8, 8, 8, 8, 8, 2, 2)
assert sum(CHUNKS) == KT
XHEAD = 8                    # x tiles shipped in the first x piece
SINGLE_PSUM = True           # both chains in one PSUM bank -> one epilogue mul

_cached_nc = None
_last_in_maps = None


def _build():
    global _cached_nc
    if _cached_nc is not None:
        return _cached_nc
    from contextlib import ExitStack

    nc = bacc.Bacc(
        "TRN2",
        target_bir_lowering=False,
        debug=False,
        enable_asserts=False,
        num_devices=NCORES,
    )
    # x SBUF image: partition p, free k*64+b = x[b, k*128+p]
    xsb = nc.dram_tensor("xsb", [128, KT * B], mybir.dt.float16, kind="ExternalInput")
    # weight codes: partition p, free k*OSH+o = fp8_code[o_shard, k*128+p]
    w8 = nc.dram_tensor("w8", [128, KT * OSH], mybir.dt.float8e3, kind="ExternalInput")
    # per-output-column scale, col-tiled broadcast: sb2[h*64+b, o'] = s[h*512+o']
    sb2 = nc.dram_tensor("sb2", [128, 512], mybir.dt.float16, kind="ExternalInput")
    out = nc.dram_tensor("out", [B, OSH], mybir.dt.float16, kind="ExternalOutput")

    ck = []
    k0 = 0
    for n in CHUNKS:
        ck.append((k0, k0 + n))
        k0 += n

    with ExitStack() as ctx:
        ec = ctx.enter_context
        dws = [ec(nc.semaphore(f"dw{i}")) for i in range(len(CHUNKS))]
        dxh = ec(nc.semaphore("dxh"))
        dxr = ec(nc.semaphore("dxr"))
        dsb = ec(nc.semaphore("dsb"))
        dout0 = ec(nc.semaphore("dout0"))
        dout1 = ec(nc.semaphore("dout1"))
        mmd1 = ec(nc.semaphore("mmd1"))
        mmd2 = ec(nc.semaphore("mmd2"))
        epi = ec(nc.semaphore("epi"))
        xt = ec(nc.sbuf_tensor("xt", [128, KT * B], mybir.dt.float16))
        w8t = ec(nc.sbuf_tensor("w8t", [128, KT * OSH], mybir.dt.float8e3))
        sbt = ec(nc.sbuf_tensor("sbt", [128, 512], mybir.dt.float16))
        o16 = ec(nc.sbuf_tensor("o16", [128, 512], mybir.dt.float16))
        ps1 = ec(nc.psum_tensor("ps1", [128, 512], mybir.dt.float32))
        if not SINGLE_PSUM:
            ps2 = ec(nc.psum_tensor("ps2", [128, 512], mybir.dt.float32))
        else:
            ps2 = ps1
        block = ec(nc.Block())

        @block.scalar
        def _(scalar):
            # This ring reaches user code earliest; kick off the first data.
            scalar.dma_start(
                xt[:, 0 : XHEAD * B], xsb[:, 0 : XHEAD * B]
            ).then_inc(dxh, 16)
            ka, kb = ck[0]
            scalar.dma_start(
                w8t[:, ka * OSH : kb * OSH], w8[:, ka * OSH : kb * OSH]
            ).then_inc(dws[0], 16)

        @block.sync
        def _(sync):
            sync.dma_start(xt[:, XHEAD * B :], xsb[:, XHEAD * B :]).then_inc(dxr, 16)
            for i, (ka, kb) in enumerate(ck[1:], start=1):
                sync.dma_start(
                    w8t[:, ka * OSH : kb * OSH], w8[:, ka * OSH : kb * OSH]
                ).then_inc(dws[i], 16)
            sync.dma_start(sbt[:, :], sb2[:, :]).then_inc(dsb, 16)
            sync.wait_ge(epi, 1)
            sync.dma_start(out[:, 0:512], o16[0:64, :]).then_inc(dout0, 16)
            sync.wait_ge(epi, 2)
            sync.dma_start(out[:, 512:1024], o16[64:128, :]).then_inc(dout1, 16)
            sync.wait_ge(dout0, 16)
            sync.wait_ge(dout1, 16)

        @block.vector
        def _(vector):
            vector.wait_ge(mmd1, 1)
            vector.wait_ge(dsb, 16)
            vector.tensor_mul(o16[0:64, :], ps1[0:64, :], sbt[0:64, :]).then_inc(
                epi, 1
            )
            vector.wait_ge(mmd2, 1)
            vector.tensor_mul(
                o16[64:128, :], ps2[64:128, :], sbt[64:128, :]
            ).then_inc(epi, 1)

        @block.tensor
        def _(tensor):
            tensor.wait_ge(dxh, 16)
            last_chunk = len(ck) - 1
            for i, (ka, kb) in enumerate(ck):
                tensor.wait_ge(dws[i], 16)
                if ka <= XHEAD < kb or ka == XHEAD:
                    tensor.wait_ge(dxr, 16)
                if i == last_chunk:
                    # Close chain 1 first so its epilogue overlaps chain 2.
                    for half, sem in ((0, mmd1), (1, mmd2)):
                        for k in range(ka, kb):
                            lhsT = xt[:, k * B : (k + 1) * B]
                            rhs = w8t[
                                :, k * OSH + half * 512 : k * OSH + (half + 1) * 512
                            ]
                            psd = ps1[0:64, :] if half == 0 else ps2[64:128, :]
                            mm = tensor.matmul(
                                psd, lhsT, rhs, start=False, stop=(k == kb - 1),
                                skip_group_check=SINGLE_PSUM,
                            )
                        mm.then_inc(sem, 1)
                else:
                    for k in range(ka, kb):
                        lhsT = xt[:, k * B : (k + 1) * B]
                        rhs = w8t[:, k * OSH : (k + 1) * OSH]
                        first = k == 0
                        tensor.matmul(
                            ps1[0:64, :], lhsT, rhs[:, 0:512], start=first,
                            stop=False, skip_group_check=SINGLE_PSUM,
                        )
                        tensor.matmul(
                            ps2[64:128, :], lhsT, rhs[:, 512:1024],
                            start=first, stop=False, skip_group_check=SINGLE_PSUM,
                        )

    nc.compile()
    _cached_nc = nc
    return nc


def kernel(x, qweight, lut):
    x = np.asarray(x, dtype=np.float16)
    qweight = np.asarray(qweight, dtype=np.int32)
    lut = np.asarray(lut, dtype=np.float16)

    # Per-row fp8 e3m4 re-encode of the LUT (scale maps row max to 15).
    lut32 = lut.astype(np.float32)
    s = np.abs(lut32).max(axis=1) / 15.0
    s[s == 0] = 1.0
    lut8 = (lut32 / s[:, None]).astype(ml_dtypes.float8_e3m4)

    # Per-element weight codes (gather as raw uint8 bit patterns).
    codes = np.take_along_axis(lut8.view(np.uint8), qweight, axis=1)  # [OUT, IN]

    # x SBUF image.
    xsb = np.ascontiguousarray(
        np.ascontiguousarray(x.T).reshape(KT, 128, B).transpose(1, 0, 2).reshape(
            128, KT * B
        )
    )

    s16 = s.astype(np.float16)
    in_maps = []
    for c in range(NCORES):
        sl = slice(c * OSH, (c + 1) * OSH)
        wt = codes[sl, :].T                                # [IN, OSH] view
        wimg = np.ascontiguousarray(
            wt.reshape(KT, 128, OSH).transpose(1, 0, 2)
        ).reshape(128, KT * OSH)
        sc = s16[sl]
        sb2 = np.ascontiguousarray(
            np.broadcast_to(
                sc.reshape(2, 512)[:, None, :], (2, B, 512)
            ).reshape(128, 512)
        )
        in_maps.append(
            {"xsb": xsb, "w8": wimg.view(ml_dtypes.float8_e3m4), "sb2": sb2}
        )

    global _last_in_maps
    _last_in_maps = in_maps

    nc = _build()
    res = run_bass_kernel_spmd(nc, in_maps, core_ids=list(range(NCORES)))
    return np.concatenate(
        [res.results[c]["out"] for c in range(NCORES)], axis=1
    ).astype(np.float16)


# revision 9
# speedup vs baseline: 1.0917x; 1.0917x over previous
"""AnyPrecisionLinear (4-bit LUT-quantized linear) on 8 TRN2 NeuronCores.

Reference computes:  out = x @ W.T,  W[o,i] = lut[o, qweight[o,i]]
  x: [64, 8192] fp16, qweight: [8192, 8192] int32 (values 0..15),
  lut: [8192, 16] fp16  ->  out: [64, 8192] fp16

Strategy (tensor-parallel along out_features, per the sharding hint):
  * Host re-encodes each row's 16-entry LUT into fp8 e3m4 (FP8_EXP3) code
    values with a per-row scale s[o] = max|lut|/15, then gathers per-element
    codes so each device receives a [128, 64*1024] fp8 weight image
    (1 byte/weight; encode rel err measured 1.09e-2 vs the 2e-2 threshold).
  * The TensorEngine consumes fp8e3 rhs directly against the fp16 x lhsT
    (mixed-dtype matmul, HW-verified bit-exact) -- NO on-device dequant.
    The kernel is a pure DMA-stream + matmul-chase pipeline.
  * All input streams ride the Sync HWDGE ring (the Scalar ring measured
    several us slower to deliver); full 128-partition transfers only
    (partition-sliced DMAs fragment across engines and halve bandwidth).
  * x ships in two pieces (head covers the first chunks) so the first
    matmul fires right after the first weight chunk lands; a short warmup
    matmul burst keeps the PE from falling behind the stream while cold.
  * Column-tiled matmul pairs accumulate into ONE PSUM bank (partitions
    0-63 chain 1 / 64-127 chain 2; start=True clears per-element, so
    disjoint-partition chains coexist -- HW-verified).  The epilogue is
    then a single full-width DVE multiply (DVE time scales with columns,
    not partitions), followed by the two output-half DMAs.
"""

import numpy as np
import ml_dtypes

import concourse.bass as bass
from concourse import bacc, mybir
from concourse.bass_utils import run_bass_kernel_spmd

B, IN, OUT, NCORES = 64, 8192, 8192, 8
OSH = OUT // NCORES          # 1024 output columns per core
KT = IN // 128               # 64 contraction tiles of 128
CHUNKS = (2, 2, 8, 8, 8, 8, 8, 8, 8, 2, 2)
assert sum(CHUNKS) == KT
XHEAD = 8                    # x tiles shipped in the first x piece
WARMUP = 48

_cached_nc = None
_last_in_maps = None


def _build():
    global _cached_nc
    if _cached_nc is not None:
        return _cached_nc
    from contextlib import ExitStack

    nc = bacc.Bacc(
        "TRN2",
        target_bir_lowering=False,
        debug=False,
        enable_asserts=False,
        num_devices=NCORES,
    )
    # x SBUF image: partition p, free k*64+b = x[b, k*128+p]
    xsb = nc.dram_tensor("xsb", [128, KT * B], mybir.dt.float16, kind="ExternalInput")
    # weight codes: partition p, free k*OSH+o = fp8_code[o_shard, k*128+p]
    w8 = nc.dram_tensor("w8", [128, KT * OSH], mybir.dt.float8e3, kind="ExternalInput")
    # per-output-column scale, col-tiled broadcast: sb2[h*64+b, o'] = s[h*512+o']
    sb2 = nc.dram_tensor("sb2", [128, 512], mybir.dt.float16, kind="ExternalInput")
    out = nc.dram_tensor("out", [B, OSH], mybir.dt.float16, kind="ExternalOutput")

    ck = []
    k0 = 0
    for n in CHUNKS:
        ck.append((k0, k0 + n))
        k0 += n

    with ExitStack() as ctx:
        ec = ctx.enter_context
        dws = [ec(nc.semaphore(f"dw{i}")) for i in range(len(CHUNKS))]
        dxh = ec(nc.semaphore("dxh"))
        dxr = ec(nc.semaphore("dxr"))
        dsb = ec(nc.semaphore("dsb"))
        dout0 = ec(nc.semaphore("dout0"))
        dout1 = ec(nc.semaphore("dout1"))
        mmd = ec(nc.semaphore("mmd"))
        epi = ec(nc.semaphore("epi"))
        wzs = ec(nc.semaphore("wzs"))
        xt = ec(nc.sbuf_tensor("xt", [128, KT * B], mybir.dt.float16))
        w8t = ec(nc.sbuf_tensor("w8t", [128, KT * OSH], mybir.dt.float8e3))
        sbt = ec(nc.sbuf_tensor("sbt", [128, 512], mybir.dt.float16))
        o16 = ec(nc.sbuf_tensor("o16", [128, 512], mybir.dt.float16))
        wz = ec(nc.sbuf_tensor("wz", [128, 32], mybir.dt.float16))
        ps1 = ec(nc.psum_tensor("ps1", [128, 512], mybir.dt.float32))
        wps = ec(nc.psum_tensor("wps", [32, 32], mybir.dt.float32))
        block = ec(nc.Block())

        @block.sync
        def _(sync):
            sync.dma_start(
                xt[:, 0 : XHEAD * B], xsb[:, 0 : XHEAD * B]
            ).then_inc(dxh, 16)
            first = True
            for i, (ka, kb) in enumerate(ck):
                sync.dma_start(
                    w8t[:, ka * OSH : kb * OSH], w8[:, ka * OSH : kb * OSH]
                ).then_inc(dws[i], 16)
                if first:
                    sync.dma_start(
                        xt[:, XHEAD * B :], xsb[:, XHEAD * B :]
                    ).then_inc(dxr, 16)
                    first = False
            sync.wait_ge(epi, 1)
            sync.dma_start(out[:, 0:512], o16[0:64, :]).then_inc(dout0, 16)
            sync.dma_start(out[:, 512:1024], o16[64:128, :]).then_inc(dout1, 16)
            sync.wait_ge(dout0, 16)
            sync.wait_ge(dout1, 16)

        @block.scalar
        def _(scalar):
            scalar.dma_start(sbt[:, :], sb2[:, :]).then_inc(dsb, 16)

        @block.gpsimd
        def _(gpsimd):
            gpsimd.memset(wz[:, :], 0).then_inc(wzs, 1)

        @block.vector
        def _(vector):
            vector.wait_ge(mmd, 1)
            vector.wait_ge(dsb, 16)
            vector.tensor_mul(o16[:, :], ps1[:, :], sbt[:, :]).then_inc(epi, 1)

        @block.tensor
        def _(tensor):
            tensor.wait_ge(wzs, 1)
            for _ in range(WARMUP):
                tensor.matmul(wps.ap(), wz[:, :], wz[:, :], start=True, stop=True)
            tensor.wait_ge(dxh, 16)
            for i, (ka, kb) in enumerate(ck):
                tensor.wait_ge(dws[i], 16)
                if ka <= XHEAD < kb or ka == XHEAD:
                    tensor.wait_ge(dxr, 16)
                for k in range(ka, kb):
                    lhsT = xt[:, k * B : (k + 1) * B]
                    rhs = w8t[:, k * OSH : (k + 1) * OSH]
                    first = k == 0
                    last = k == KT - 1
                    tensor.matmul(
                        ps1[0:64, :], lhsT, rhs[:, 0:512], start=first,
                        stop=last, skip_group_check=True,
                    )
                    mm2 = tensor.matmul(
                        ps1[64:128, :], lhsT, rhs[:, 512:1024],
                        start=first, stop=last, skip_group_check=True,
                    )
                    if last:
                        mm2.then_inc(mmd, 1)

    nc.compile()
    _cached_nc = nc
    return nc


def kernel(x, qweight, lut):
    x = np.asarray(x, dtype=np.float16)
    qweight = np.asarray(qweight, dtype=np.int32)
    lut = np.asarray(lut, dtype=np.float16)

    # Per-row fp8 e3m4 re-encode of the LUT (scale maps row max to 15).
    lut32 = lut.astype(np.float32)
    s = np.abs(lut32).max(axis=1) / 15.0
    s[s == 0] = 1.0
    lut8 = (lut32 / s[:, None]).astype(ml_dtypes.float8_e3m4)

    # Per-element weight codes (gather as raw uint8 bit patterns).
    codes = np.take_along_axis(lut8.view(np.uint8), qweight, axis=1)  # [OUT, IN]

    # x SBUF image.
    xsb = np.ascontiguousarray(
        np.ascontiguousarray(x.T).reshape(KT, 128, B).transpose(1, 0, 2).reshape(
            128, KT * B
        )
    )

    s16 = s.astype(np.float16)
    in_maps = []
    for c in range(NCORES):
        sl = slice(c * OSH, (c + 1) * OSH)
        wt = codes[sl, :].T                                # [IN, OSH] view
        wimg = np.ascontiguousarray(
            wt.reshape(KT, 128, OSH).transpose(1, 0, 2)
        ).reshape(128, KT * OSH)
        sc = s16[sl]
        sb2 = np.ascontiguousarray(
            np.broadcast_to(
                sc.reshape(2, 512)[:, None, :], (2, B, 512)
            ).reshape(128, 512)
        )
        in_maps.append(
            {"xsb": xsb, "w8": wimg.view(ml_dtypes.float8_e3m4), "sb2": sb2}
        )

    global _last_in_maps
    _last_in_maps = in_maps

    nc = _build()
    res = run_bass_kernel_spmd(nc, in_maps, core_ids=list(range(NCORES)))
    return np.concatenate(
        [res.results[c]["out"] for c in range(NCORES)], axis=1
    ).astype(np.float16)


# revision 10
# speedup vs baseline: 1.0925x; 1.0008x over previous
"""AnyPrecisionLinear (4-bit LUT-quantized linear) on 8 TRN2 NeuronCores.

Reference computes:  out = x @ W.T,  W[o,i] = lut[o, qweight[o,i]]
  x: [64, 8192] fp16, qweight: [8192, 8192] int32 (values 0..15),
  lut: [8192, 16] fp16  ->  out: [64, 8192] fp16

Strategy (tensor-parallel along out_features, per the sharding hint):
  * Host re-encodes each row's 16-entry LUT into fp8 e3m4 (FP8_EXP3) code
    values with a per-row scale s[o] = max|lut|/15, then gathers per-element
    codes so each device receives a [128, 64*1024] fp8 weight image
    (1 byte/weight; encode rel err measured 1.09e-2 vs the 2e-2 threshold).
  * The TensorEngine consumes fp8e3 rhs directly against the fp16 x lhsT
    (mixed-dtype matmul, HW-verified bit-exact) -- NO on-device dequant.
    The kernel is a pure DMA-stream + matmul-chase pipeline on the Sync
    HWDGE ring (the Scalar ring measured several us slower; partition-
    sliced DMAs fragment descriptors and halve bandwidth, so all transfers
    are full 128-partition).
  * The per-output-column fp16 scale row rides appended to the fp8 weight
    image (bitcast slice on-device), so it needs no separate transfer or
    semaphore and lands with the last chunk.
  * Semaphores are expensive to tear down (~0.4us each, all-engine
    barrier per sem at block close), so the chunk count and sem count are
    kept minimal: 12 sems total.
  * x ships in two pieces (head covers the first chunks) so the first
    matmul fires right after the first weight chunk lands; a short warmup
    matmul burst (on uninitialized scratch -- output discarded) keeps the
    PE from falling too far behind the stream while HAM-cold.
  * Column-tiled matmul pairs accumulate into ONE PSUM bank (partitions
    0-63 / 64-127; start=True clears per-element, so disjoint-partition
    chains coexist -- HW-verified), then a single full-width DVE multiply
    applies the scale and the two output-half DMAs drain.
"""

import numpy as np
import ml_dtypes

import concourse.bass as bass
from concourse import bacc, mybir
from concourse.bass_utils import run_bass_kernel_spmd

B, IN, OUT, NCORES = 64, 8192, 8192, 8
OSH = OUT // NCORES          # 1024 output columns per core
KT = IN // 128               # 64 contraction tiles of 128
CHUNKS = (2, 2, 16, 16, 16, 10, 2)
assert sum(CHUNKS) == KT
XHEAD = 8                    # x tiles shipped in the first x piece
WARMUP = 48
WTAIL = 1024                 # fp8-elem tail of the weight image holding sb2

_cached_nc = None
_last_in_maps = None


def _build():
    global _cached_nc
    if _cached_nc is not None:
        return _cached_nc
    from contextlib import ExitStack

    nc = bacc.Bacc(
        "TRN2",
        target_bir_lowering=False,
        debug=False,
        enable_asserts=False,
        num_devices=NCORES,
    )
    # x SBUF image: partition p, free k*64+b = x[b, k*128+p]
    xsb = nc.dram_tensor("xsb", [128, KT * B], mybir.dt.float16, kind="ExternalInput")
    # weight codes: partition p, free k*OSH+o = fp8_code[o_shard, k*128+p];
    # the last WTAIL fp8 elems are the fp16 scale image sb2, bit-packed.
    w8 = nc.dram_tensor(
        "w8", [128, KT * OSH + WTAIL], mybir.dt.float8e3, kind="ExternalInput"
    )
    out = nc.dram_tensor("out", [B, OSH], mybir.dt.float16, kind="ExternalOutput")

    ck = []
    k0 = 0
    for n in CHUNKS:
        ck.append((k0, k0 + n))
        k0 += n

    with ExitStack() as ctx:
        ec = ctx.enter_context
        dws = [ec(nc.semaphore(f"dw{i}")) for i in range(len(CHUNKS))]
        dxh = ec(nc.semaphore("dxh"))
        dxr = ec(nc.semaphore("dxr"))
        dout = ec(nc.semaphore("dout"))
        mmd = ec(nc.semaphore("mmd"))
        epi = ec(nc.semaphore("epi"))
        xt = ec(nc.sbuf_tensor("xt", [128, KT * B], mybir.dt.float16))
        w8t = ec(
            nc.sbuf_tensor("w8t", [128, KT * OSH + WTAIL], mybir.dt.float8e3)
        )
        o16 = ec(nc.sbuf_tensor("o16", [128, 512], mybir.dt.float16))
        wz = ec(nc.sbuf_tensor("wz", [128, 32], mybir.dt.float16))
        ps1 = ec(nc.psum_tensor("ps1", [128, 512], mybir.dt.float32))
        wps = ec(nc.psum_tensor("wps", [32, 32], mybir.dt.float32))
        block = ec(nc.Block())

        @block.sync
        def _(sync):
            sync.dma_start(
                xt[:, 0 : XHEAD * B], xsb[:, 0 : XHEAD * B]
            ).then_inc(dxh, 16)
            first = True
            for i, (ka, kb) in enumerate(ck):
                hi = kb * OSH + (WTAIL if i == len(ck) - 1 else 0)
                sync.dma_start(
                    w8t[:, ka * OSH : hi], w8[:, ka * OSH : hi]
                ).then_inc(dws[i], 16)
                if first:
                    sync.dma_start(
                        xt[:, XHEAD * B :], xsb[:, XHEAD * B :]
                    ).then_inc(dxr, 16)
                    first = False
            sync.wait_ge(epi, 1)
            sync.dma_start(out[:, 0:512], o16[0:64, :]).then_inc(dout, 16)
            sync.dma_start(out[:, 512:1024], o16[64:128, :]).then_inc(dout, 16)
            sync.wait_ge(dout, 32)

        @block.vector
        def _(vector):
            # sb2 lands with the last weight chunk, which also gates mmd.
            sbt = w8t[:, KT * OSH : KT * OSH + WTAIL].bitcast(mybir.dt.float16)
            vector.wait_ge(mmd, 1)
            vector.tensor_mul(o16[:, :], ps1[:, :], sbt).then_inc(epi, 1)

        @block.tensor
        def _(tensor):
            for _ in range(WARMUP):
                tensor.matmul(wps.ap(), wz[:, :], wz[:, :], start=True, stop=True)
            tensor.wait_ge(dxh, 16)
            for i, (ka, kb) in enumerate(ck):
                tensor.wait_ge(dws[i], 16)
                if ka <= XHEAD < kb or ka == XHEAD:
                    tensor.wait_ge(dxr, 16)
                for k in range(ka, kb):
                    lhsT = xt[:, k * B : (k + 1) * B]
                    rhs = w8t[:, k * OSH : (k + 1) * OSH]
                    first = k == 0
                    last = k == KT - 1
                    tensor.matmul(
                        ps1[0:64, :], lhsT, rhs[:, 0:512], start=first,
                        stop=last, skip_group_check=True,
                    )
                    mm2 = tensor.matmul(
                        ps1[64:128, :], lhsT, rhs[:, 512:1024],
                        start=first, stop=last, skip_group_check=True,
                    )
                    if last:
                        mm2.then_inc(mmd, 1)

    nc.compile()
    _cached_nc = nc
    return nc


def kernel(x, qweight, lut):
    x = np.asarray(x, dtype=np.float16)
    qweight = np.asarray(qweight, dtype=np.int32)
    lut = np.asarray(lut, dtype=np.float16)

    # Per-row fp8 e3m4 re-encode of the LUT (scale maps row max to 15).
    lut32 = lut.astype(np.float32)
    s = np.abs(lut32).max(axis=1) / 15.0
    s[s == 0] = 1.0
    lut8 = (lut32 / s[:, None]).astype(ml_dtypes.float8_e3m4)

    # Per-element weight codes (gather as raw uint8 bit patterns).
    codes = np.take_along_axis(lut8.view(np.uint8), qweight, axis=1)  # [OUT, IN]

    # x SBUF image.
    xsb = np.ascontiguousarray(
        np.ascontiguousarray(x.T).reshape(KT, 128, B).transpose(1, 0, 2).reshape(
            128, KT * B
        )
    )

    s16 = s.astype(np.float16)
    in_maps = []
    for c in range(NCORES):
        sl = slice(c * OSH, (c + 1) * OSH)
        wt = codes[sl, :].T                                # [IN, OSH] view
        wimg = np.empty((128, KT * OSH + WTAIL), np.uint8)
        wimg[:, : KT * OSH] = wt.reshape(KT, 128, OSH).transpose(1, 0, 2).reshape(
            128, KT * OSH
        )
        sc = s16[sl]
        sb2 = np.broadcast_to(
            sc.reshape(2, 512)[:, None, :], (2, B, 512)
        ).reshape(128, 512)
        wimg[:, KT * OSH :] = np.ascontiguousarray(sb2).view(np.uint8)
        in_maps.append({"xsb": xsb, "w8": wimg.view(ml_dtypes.float8_e3m4)})

    global _last_in_maps
    _last_in_maps = in_maps

    nc = _build()
    res = run_bass_kernel_spmd(nc, in_maps, core_ids=list(range(NCORES)))
    return np.concatenate(
        [res.results[c]["out"] for c in range(NCORES)], axis=1
    ).astype(np.float16)
